# revision 1
# baseline (speedup 1.0000x reference)
"""Trainium2 Bass kernel for GaussianFPSPooling.

Pipeline (per batch element, one NeuronCore):
  1. Farthest-point sampling over N=100000 3-D points, K=256 iterations,
     fully SBUF-resident.  Arithmetic replicates the jax-CPU reference
     bit-exactly ((x-px)^2 + (y-py)^2) + (z-pz)^2, f32, left-assoc, min
     accumulate, first-index argmax) so the selected indices match.
  2. Indirect-DMA gather of the 256 selected feature rows from HBM.
  3. PE transpose + matmul with W (f32) + bias.

Distribution: data-parallel over the batch (B=4) across 8 cores; cores
c and c+4 run the same batch (c % 4), host reads cores 0-3.
"""

import sys

if "/opt/trn_rl_repo" not in sys.path:
    sys.path.insert(0, "/opt/trn_rl_repo")

import numpy as np

import concourse.bacc as bacc
import concourse.bass as bass
import concourse.bass_isa as bass_isa
import concourse.mybir as mybir
from concourse import tile
from concourse.bass_utils import run_bass_kernel_spmd

F32 = mybir.dt.float32
I32 = mybir.dt.int32
Alu = mybir.AluOpType
Act = mybir.ActivationFunctionType

# problem sizes (hardcoded per contract)
B = 4
N = 100000
D_IN = 128
D_OUT = 256
K = 256
P = 128               # partitions
BIGI = float(1 << 20)  # index-encoding base: stores BIGI - idx (exact in f32)


def _ceil_div(a, b):
    return (a + b - 1) // b


def build_fps_kernel(n=N, k=K, d_in=D_IN, d_out=D_OUT, with_linear=True):
    """Build the Bass program. Returns (nc, C) with C = cols per partition."""
    C = _ceil_div(n, P)
    npad = P * C

    nc = bacc.Bacc("TRN2", target_bir_lowering=False)

    # ---- DRAM I/O ----
    xs_d = nc.dram_tensor("xs", [P, C], F32, kind="ExternalInput")
    ys_d = nc.dram_tensor("ys", [P, C], F32, kind="ExternalInput")
    zs_d = nc.dram_tensor("zs", [P, C], F32, kind="ExternalInput")
    g2_d = nc.dram_tensor("g2", [P, C], F32, kind="ExternalInput")
    dists_d = nc.dram_tensor("dists0", [P, C], F32, kind="ExternalInput")
    pt0_d = nc.dram_tensor("pt0", [P, 4], F32, kind="ExternalInput")
    idx_d = nc.dram_tensor("idx_out", [1, k], F32, kind="ExternalOutput")
    if with_linear:
        feat_d = nc.dram_tensor("feat", [n, d_in], F32, kind="ExternalInput")
        w_d = nc.dram_tensor("w", [d_in, d_out], F32, kind="ExternalInput")
        brow_d = nc.dram_tensor("brow", [1, d_out], F32, kind="ExternalInput")
        ones1_d = nc.dram_tensor("ones1", [1, P], F32, kind="ExternalInput")
        ident_d = nc.dram_tensor("ident", [P, P], F32, kind="ExternalInput")
        out_d = nc.dram_tensor("out", [k, d_out], F32, kind="ExternalOutput")

    kg = k // P if with_linear else 0  # gather column groups
    if with_linear:
        assert k % P == 0

    # position of iteration-k index inside idxraw (so a plain [1,k]->[P,kg]
    # SBUF->SBUF DMA lands index of sample k at partition k%P, col k//P)
    if with_linear:
        pos = [(kk % P) * kg + (kk // P) for kk in range(k)]
    else:
        pos = list(range(k))

    with tile.TileContext(nc) as tc:
        with (
            tc.tile_pool(name="const", bufs=1) as cp,
            tc.tile_pool(name="loop", bufs=2) as lp,
            tc.tile_pool(name="psum", bufs=2, space="PSUM") as pp,
        ):
            xs = cp.tile([P, C], F32, tag="xs")
            ys = cp.tile([P, C], F32, tag="ys")
            zs = cp.tile([P, C], F32, tag="zs")
            g2 = cp.tile([P, C], F32, tag="g2")
            dists = cp.tile([P, C], F32, tag="dists")
            pt0 = cp.tile([P, 4], F32, tag="pt0")
            idxraw = cp.tile([1, k], F32, tag="idxraw")

            nc.sync.dma_start(xs[:], xs_d[:])
            nc.sync.dma_start(ys[:], ys_d[:])
            nc.sync.dma_start(zs[:], zs_d[:])
            nc.sync.dma_start(g2[:], g2_d[:])
            nc.sync.dma_start(dists[:], dists_d[:])
            nc.sync.dma_start(pt0[:], pt0_d[:])
            nc.vector.memset(idxraw[:], BIGI)  # sample 0 is point 0

            pt = pt0
            for it in range(k - 1):
                px = pt[:, 0:1]
                py = pt[:, 1:2]
                pz = pt[:, 2:3]
                # d = ((x-px)^2 + (y-py)^2) + (z-pz)^2, bit-exact f32
                t1 = lp.tile([P, C], F32, tag="t1")
                nc.scalar.activation(t1[:], xs[:], Act.Square, bias=px, scale=-1.0)
                t2 = lp.tile([P, C], F32, tag="t2")
                nc.scalar.activation(t2[:], ys[:], Act.Square, bias=py, scale=-1.0)
                t3 = lp.tile([P, C], F32, tag="t3")
                nc.scalar.activation(t3[:], zs[:], Act.Square, bias=pz, scale=-1.0)
                s = lp.tile([P, C], F32, tag="s")
                nc.vector.tensor_tensor(s[:], t1[:], t2[:], op=Alu.add)
                nc.vector.tensor_tensor(s[:], s[:], t3[:], op=Alu.add)
                # dists = min(dists, d); permax = rowwise max of new dists
                # (tensor_tensor_reduce would fuse these but crashes this
                # runtime, so keep them split)
                permax = lp.tile([P, 1], F32, tag="permax")
                nc.vector.tensor_tensor(dists[:], dists[:], s[:], op=Alu.min)
                nc.vector.reduce_max(permax[:], dists[:], axis=mybir.AxisListType.X)
                gmax = lp.tile([P, 1], F32, tag="gmax")
                nc.gpsimd.partition_all_reduce(
                    gmax[:], permax[:], channels=P, reduce_op=bass_isa.ReduceOp.max
                )
                # encode argmax as max over (dists==gmax)*(BIGI-idx)
                mi = lp.tile([P, C], F32, tag="mi")
                nc.vector.scalar_tensor_tensor(
                    mi[:], in0=dists[:], scalar=gmax[:], in1=g2[:],
                    op0=Alu.is_equal, op1=Alu.mult,
                )
                permax2 = lp.tile([P, 1], F32, tag="permax2")
                nc.vector.reduce_max(permax2[:], mi[:], axis=mybir.AxisListType.X)
                is2 = lp.tile([P, 1], F32, tag="is2")
                nc.gpsimd.partition_all_reduce(
                    is2[:], permax2[:], channels=P, reduce_op=bass_isa.ReduceOp.max
                )
                # record BIGI - idx (decoded after the loop)
                nc.scalar.copy(idxraw[0:1, pos[it + 1] : pos[it + 1] + 1],
                               is2[0:1, 0:1])
                # extract winner coords: one-hot (g2==is2) dot each plane
                ptn = lp.tile([P, 4], F32, tag="ptn")
                junk = lp.tile([P, C], F32, tag="junk")
                nc.vector.scalar_tensor_tensor(
                    junk[:], in0=g2[:], scalar=is2[:], in1=xs[:],
                    op0=Alu.is_equal, op1=Alu.mult, accum_out=ptn[:, 0:1],
                )
                nc.vector.scalar_tensor_tensor(
                    junk[:], in0=g2[:], scalar=is2[:], in1=ys[:],
                    op0=Alu.is_equal, op1=Alu.mult, accum_out=ptn[:, 1:2],
                )
                nc.vector.scalar_tensor_tensor(
                    junk[:], in0=g2[:], scalar=is2[:], in1=zs[:],
                    op0=Alu.is_equal, op1=Alu.mult, accum_out=ptn[:, 2:3],
                )
                ptb = lp.tile([P, 4], F32, tag="ptb")
                nc.gpsimd.partition_all_reduce(
                    ptb[:, 0:3], ptn[:, 0:3], channels=P,
                    reduce_op=bass_isa.ReduceOp.add,
                )
                pt = ptb

            # decode indices: idx = BIGI - idxraw
            idxf = cp.tile([1, k], F32, tag="idxf")
            nc.vector.tensor_scalar(
                idxf[:], idxraw[:], -1.0, BIGI, op0=Alu.mult, op1=Alu.add
            )
            nc.sync.dma_start(idx_d[:], idxf[:])

            if with_linear:
                w_sb = cp.tile([d_in, d_out], F32, tag="w")
                brow = cp.tile([1, d_out], F32, tag="brow")
                ones1 = cp.tile([1, P], F32, tag="ones1")
                ident = cp.tile([P, P], F32, tag="ident")
                nc.sync.dma_start(w_sb[:], w_d[:])
                nc.sync.dma_start(brow[:], brow_d[:])
                nc.sync.dma_start(ones1[:], ones1_d[:])
                nc.sync.dma_start(ident[:], ident_d[:])

                idxi = cp.tile([1, k], I32, tag="idxi")
                nc.vector.tensor_copy(idxi[:], idxf[:])
                gidx = cp.tile([P, kg], I32, tag="gidx")
                nc.sync.dma_start(gidx[:], idxi[:])  # relayout [1,k]->[P,kg]

                for j in range(kg):
                    gath = cp.tile([P, d_in], F32, tag=f"gath{j}")
                    nc.gpsimd.indirect_dma_start(
                        out=gath[:],
                        out_offset=None,
                        in_=feat_d[:],
                        in_offset=bass.IndirectOffsetOnAxis(
                            ap=gidx[:, j : j + 1], axis=0
                        ),
                    )
                    tp_ps = pp.tile([P, P], F32, tag="tp")
                    nc.tensor.transpose(tp_ps[:], gath[:], ident[:])
                    lhsT = cp.tile([P, P], F32, tag=f"lhsT{j}")
                    nc.vector.tensor_copy(lhsT[:], tp_ps[:])
                    out_ps = pp.tile([P, d_out], F32, tag="outps")
                    nc.tensor.matmul(
                        out_ps[:], lhsT=lhsT[:], rhs=w_sb[:], start=True, stop=False
                    )
                    nc.tensor.matmul(
                        out_ps[:], lhsT=ones1[:], rhs=brow[:], start=False, stop=True
                    )
                    outt = cp.tile([P, d_out], F32, tag=f"outt{j}")
                    nc.vector.tensor_copy(outt[:], out_ps[:])
                    nc.sync.dma_start(out_d[j * P : (j + 1) * P, :], outt[:])

    nc.compile()
    return nc, C


def make_core_inputs(means_b, features_b=None, W=None, bvec=None,
                     n=N, k=K, with_linear=True):
    """Host-side layout for one batch element."""
    C = _ceil_div(n, P)
    npad = P * C
    m = np.asarray(means_b, np.float32)
    planes = np.zeros((npad, 3), np.float32)
    planes[:n] = m
    d0 = np.full(npad, -1.0, np.float32)
    d0[:n] = np.inf
    g2 = np.zeros(npad, np.float32)
    g2[:n] = BIGI - np.arange(n, dtype=np.float32)
    pt0 = np.zeros((P, 4), np.float32)
    pt0[:, 0:3] = m[0]
    d = {
        "xs": planes[:, 0].reshape(P, C).copy(),
        "ys": planes[:, 1].reshape(P, C).copy(),
        "zs": planes[:, 2].reshape(P, C).copy(),
        "g2": g2.reshape(P, C).copy(),
        "dists0": d0.reshape(P, C).copy(),
        "pt0": pt0,
    }
    if with_linear:
        d["feat"] = np.ascontiguousarray(features_b, dtype=np.float32)
        d["w"] = np.ascontiguousarray(W, dtype=np.float32)
        d["brow"] = np.ascontiguousarray(bvec, dtype=np.float32).reshape(1, -1)
        d["ones1"] = np.ones((1, P), np.float32)
        d["ident"] = np.eye(P, dtype=np.float32)
    return d


_CACHE = {}


def _get_kernel():
    if "nc" not in _CACHE:
        _CACHE["nc"] = build_fps_kernel()[0]
    return _CACHE["nc"]


def kernel(features, means, W, b, trace=False):
    features = np.asarray(features, np.float32)
    means = np.asarray(means, np.float32)
    W = np.asarray(W, np.float32)
    b = np.asarray(b, np.float32)

    nc = _get_kernel()
    in_maps = []
    for c in range(8):
        bb = c % B
        in_maps.append(make_core_inputs(means[bb], features[bb], W, b))
    import time as _time

    t0 = _time.time()
    res = run_bass_kernel_spmd(nc, in_maps, core_ids=list(range(8)), trace=trace)
    _CACHE["last_run_s"] = _time.time() - t0
    out = np.stack([res.results[bb]["out"] for bb in range(B)], axis=0)
    _CACHE["last_results"] = res
    return out


if __name__ == "__main__":
    ins = dict(np.load("/tmp/inputs.npz"))
    out = kernel(**ins)
    print("out", out.shape, out.dtype)



# revision 2
# speedup vs baseline: 1.9506x; 1.9506x over previous
"""Trainium2 Bass kernel for GaussianFPSPooling — v5 (hardware loop).

Cost model measured on this runtime: cross-engine semaphore handoffs are
~100-250 us each; the v2 loop had ~7 per iteration (1.75 ms/iter).  v4
restructures each FPS iteration into exactly two engine blocks:

  vector block (per-partition, no global state needed):
      d = (x-px)^2+(y-py)^2+(z-pz)^2 ; dists = min(dists, d)
      permax[p]  = rowwise max of dists          (per-partition max)
      perenc[p]  = rowwise max of (dists==permax)*(BIGI-idx)
                   (encoded first-index argmax WITHIN partition p)
      cx/cy/cz[p] = one-hot(g2==perenc) . xs/ys/zs  (candidate coords)

  gpsimd block (global, one visit):
      gmax = partition_all_reduce(permax, max)
      e    = (permax==gmax) * perenc        ; enc = all_reduce(e, max)
      hot  = (perenc==enc)                  ; sel = hot * (cx,cy,cz)
      pt   = partition_all_reduce(sel, add) ; record enc

Tie-breaking is EXACT (matches jnp.argmax first-index): per-partition
index ranges are disjoint and ordered, so max over masked perenc picks
the smallest global index among all positions achieving gmax, and
(perenc==enc) is a unique one-hot across partitions.

Distribution: data-parallel over batch (B=4) across 8 cores; cores c and
c+4 run the same batch (c % 4); host reads cores 0-3.  Host does the
256-row gather + small linear (bit-clean f32).
"""

import sys

if "/opt/trn_rl_repo" not in sys.path:
    sys.path.insert(0, "/opt/trn_rl_repo")

import numpy as np

import concourse.bacc as bacc
import concourse.bass as bass
import concourse.bass_isa as bass_isa
import concourse.mybir as mybir
from concourse import tile
from concourse.bass_utils import run_bass_kernel_spmd

F32 = mybir.dt.float32
I32 = mybir.dt.int32
Alu = mybir.AluOpType
Act = mybir.ActivationFunctionType

# problem sizes (hardcoded per contract)
B = 4
N = 100000
D_IN = 128
D_OUT = 256
K = 256
P = 128               # partitions
BIGI = float(1 << 20)  # index-encoding base: stores BIGI - idx (exact in f32)


def _ceil_div(a, b):
    return (a + b - 1) // b


C = _ceil_div(N, P)


def build_fps_kernel(n=N, k=K, use_scalar_square=True):
    """FPS-only Bass program: one packed input, selected indices out."""
    c = _ceil_div(n, P)

    nc = bacc.Bacc("TRN2", target_bir_lowering=False)

    # packed input: [:, 0:c]=xs, [:, c:2c]=ys, [:, 2c:3c]=zs, [:, 3c:3c+4]=pt0
    in_d = nc.dram_tensor("inp", [P, 3 * c + 4], F32, kind="ExternalInput")
    idx_d = nc.dram_tensor("idx_out", [1, k], F32, kind="ExternalOutput")

    g = nc.gpsimd
    v = nc.vector

    with tile.TileContext(nc) as tc:
        with tc.tile_pool(name="const", bufs=1) as cp:
            inp = cp.tile([P, 3 * c + 4], F32, tag="inp")
            nc.sync.dma_start(inp[:], in_d[:])
            xs = inp[:, 0:c]
            ys = inp[:, c : 2 * c]
            zs = inp[:, 2 * c : 3 * c]
            pt0 = inp[:, 3 * c : 3 * c + 4]

            g2 = cp.tile([P, c], F32, tag="g2")
            dists = cp.tile([P, c], F32, tag="dists")
            idxraw = cp.tile([1, k], F32, tag="idxraw")
            v.memset(idxraw[:], BIGI)  # sample 0 is point 0

            # g2[p, j] = BIGI - (p*c + j): encoded linear index
            idxi = cp.tile([P, c], I32, tag="idxi")
            g.iota(idxi[:], pattern=[[1, c]], base=0, channel_multiplier=c)
            idxf32 = cp.tile([P, c], F32, tag="idxf32")
            v.tensor_copy(idxf32[:], idxi[:])
            v.tensor_scalar(g2[:], idxf32[:], -1.0, BIGI, op0=Alu.mult, op1=Alu.add)
            # dists0 = 1e30 valid / -1 pad (bit-identical to +inf init:
            # min(1e30, d) == d for every real squared distance)
            mask = cp.tile([P, c], F32, tag="mask")
            v.tensor_scalar(mask[:], idxf32[:], float(n), None, op0=Alu.is_lt)
            v.tensor_scalar(dists[:], mask[:], 1.0e30, -1.0, op0=Alu.mult, op1=Alu.add)

            # persistent loop state (all in-place; body traced once)
            pt = cp.tile([P, 4], F32, tag="pt")
            v.tensor_copy(pt[:], pt0)
            with tc.For_i(0, k - 1, 1) as it:
                px = pt[:, 0:1]
                py = pt[:, 1:2]
                pz = pt[:, 2:3]
                # --- vector block: distances + per-partition candidates ---
                s = cp.tile([P, c], F32, tag="s")
                if use_scalar_square:
                    # scalar engine: t = Square(-x + px) = (x-px)^2
                    t1 = cp.tile([P, c], F32, tag="t1")
                    nc.scalar.activation(t1[:], xs, Act.Square, bias=px, scale=-1.0)
                    t2 = cp.tile([P, c], F32, tag="t2")
                    nc.scalar.activation(t2[:], ys, Act.Square, bias=py, scale=-1.0)
                    t3 = cp.tile([P, c], F32, tag="t3")
                    nc.scalar.activation(t3[:], zs, Act.Square, bias=pz, scale=-1.0)
                    v.tensor_tensor(s[:], t1[:], t2[:], op=Alu.add)
                    v.tensor_tensor(s[:], s[:], t3[:], op=Alu.add)
                else:
                    tx = cp.tile([P, c], F32, tag="tx")
                    v.tensor_scalar(tx[:], xs, px, None, op0=Alu.subtract)
                    v.tensor_tensor(s[:], tx[:], tx[:], op=Alu.mult)
                    ty = cp.tile([P, c], F32, tag="ty")
                    v.tensor_scalar(ty[:], ys, py, None, op0=Alu.subtract)
                    t2 = cp.tile([P, c], F32, tag="t2")
                    v.tensor_tensor(t2[:], ty[:], ty[:], op=Alu.mult)
                    v.tensor_tensor(s[:], s[:], t2[:], op=Alu.add)
                    tz = cp.tile([P, c], F32, tag="tz")
                    v.tensor_scalar(tz[:], zs, pz, None, op0=Alu.subtract)
                    t3 = cp.tile([P, c], F32, tag="t3")
                    v.tensor_tensor(t3[:], tz[:], tz[:], op=Alu.mult)
                    v.tensor_tensor(s[:], s[:], t3[:], op=Alu.add)
                v.tensor_tensor(dists[:], dists[:], s[:], op=Alu.min)
                # cand[:,0]=permax, [:,1]=perenc, [:,2:5]=cx,cy,cz
                cand = cp.tile([P, 8], F32, tag="cand")
                v.reduce_max(cand[:, 0:1], dists[:], axis=mybir.AxisListType.X)
                mi = cp.tile([P, c], F32, tag="mi")
                v.scalar_tensor_tensor(
                    mi[:], in0=dists[:], scalar=cand[:, 0:1], in1=g2[:],
                    op0=Alu.is_equal, op1=Alu.mult,
                )
                v.reduce_max(cand[:, 1:2], mi[:], axis=mybir.AxisListType.X)
                junk = cp.tile([P, c], F32, tag="junk")
                v.scalar_tensor_tensor(
                    junk[:], in0=g2[:], scalar=cand[:, 1:2], in1=xs,
                    op0=Alu.is_equal, op1=Alu.mult, accum_out=cand[:, 2:3],
                )
                v.scalar_tensor_tensor(
                    junk[:], in0=g2[:], scalar=cand[:, 1:2], in1=ys,
                    op0=Alu.is_equal, op1=Alu.mult, accum_out=cand[:, 3:4],
                )
                v.scalar_tensor_tensor(
                    junk[:], in0=g2[:], scalar=cand[:, 1:2], in1=zs,
                    op0=Alu.is_equal, op1=Alu.mult, accum_out=cand[:, 4:5],
                )
                # --- gpsimd block: global argmax + winner payload ---
                gmax = cp.tile([P, 1], F32, tag="gmax")
                g.partition_all_reduce(
                    gmax[:], cand[:, 0:1], channels=P,
                    reduce_op=bass_isa.ReduceOp.max,
                )
                m1 = cp.tile([P, 1], F32, tag="m1")
                g.tensor_scalar(m1[:], cand[:, 0:1], gmax[:], None, op0=Alu.is_equal)
                e = cp.tile([P, 1], F32, tag="e")
                g.tensor_tensor(e[:], m1[:], cand[:, 1:2], op=Alu.mult)
                enc = cp.tile([P, 1], F32, tag="enc")
                g.partition_all_reduce(
                    enc[:], e[:], channels=P, reduce_op=bass_isa.ReduceOp.max
                )
                # record BIGI - idx (decoded after the loop)
                g.tensor_copy(idxraw[0:1, bass.ds(it + 1, 1)], enc[0:1, 0:1])
                hot = cp.tile([P, 1], F32, tag="hot")
                g.tensor_scalar(hot[:], cand[:, 1:2], enc[:], None, op0=Alu.is_equal)
                sel = cp.tile([P, 4], F32, tag="sel")
                g.tensor_scalar(sel[:, 0:3], cand[:, 2:5], hot[:], None, op0=Alu.mult)
                g.partition_all_reduce(
                    pt[:, 0:3], sel[:, 0:3], channels=P,
                    reduce_op=bass_isa.ReduceOp.add,
                )

            # decode indices: idx = BIGI - idxraw
            idxf = cp.tile([1, k], F32, tag="idxf")
            v.tensor_scalar(idxf[:], idxraw[:], -1.0, BIGI, op0=Alu.mult, op1=Alu.add)
            nc.sync.dma_start(idx_d[:], idxf[:])

    nc.compile()
    return nc, c


def make_core_inputs(means_b, n=N):
    """Packed host-side layout for one batch element."""
    c = _ceil_div(n, P)
    npad = P * c
    m = np.asarray(means_b, np.float32)
    planes = np.zeros((npad, 3), np.float32)
    planes[:n] = m
    inp = np.empty((P, 3 * c + 4), np.float32)
    inp[:, 0:c] = planes[:, 0].reshape(P, c)
    inp[:, c : 2 * c] = planes[:, 1].reshape(P, c)
    inp[:, 2 * c : 3 * c] = planes[:, 2].reshape(P, c)
    inp[:, 3 * c : 3 * c + 3] = m[0]
    inp[:, 3 * c + 3] = 0.0
    return {"inp": inp}


_CACHE = {}


def _get_kernel():
    if "nc" not in _CACHE:
        _CACHE["nc"] = build_fps_kernel()[0]
    return _CACHE["nc"]


def kernel(features, means, W, b, trace=False):
    features = np.asarray(features, np.float32)
    means = np.asarray(means, np.float32)
    W = np.asarray(W, np.float32)
    b = np.asarray(b, np.float32)

    nc = _get_kernel()
    per_batch = [make_core_inputs(means[bb]) for bb in range(B)]
    in_maps = [per_batch[c % B] for c in range(8)]
    import time as _time

    t0 = _time.time()
    res = run_bass_kernel_spmd(nc, in_maps, core_ids=list(range(8)), trace=trace)
    _CACHE["last_run_s"] = _time.time() - t0
    _CACHE["last_results"] = res

    idx = np.stack(
        [np.rint(res.results[bb]["idx_out"][0]).astype(np.int64) for bb in range(B)]
    )  # [B, K]
    _CACHE["last_idx"] = idx
    sampled = np.take_along_axis(features, idx[:, :, None], axis=1)  # [B,K,D]
    return sampled @ W + b[None, None, :]


if __name__ == "__main__":
    ins = dict(np.load("/tmp/inputs.npz"))
    out = kernel(**ins)
    print("out", out.shape, out.dtype)
